# revision 21
# baseline (speedup 1.0000x reference)
"""Dilated attention (B=4,S=4096,D=768,H=12,DIL=8) on 8 TRN2 NeuronCores.

Sharding: batch x seq-half data-parallel -> core c handles batch c//2,
sequence half c%2 (2048 query tokens). The DIL-strided K/V positions
(512 per batch) are position-independent, so each core computes K/V for
its batch's 512 dilated positions locally.

V2 design notes (vs the f32r baseline; HW-measured 496us -> 319us per
iteration under the chained-repeat burst-slope protocol of hwtime.py):
- All matmul operands are bf16 (host-cast): measured identical PE rate
  to f32r (616 vs 614 ns per [128,512] matmul) at half the HBM traffic
  and SBUF footprint; f32 PSUM accumulation keeps precision (end-to-end
  rel err 3.2e-3 vs the 2e-2 gate).
- bk is dropped entirely (softmax shift invariance: (q+bq)dot(k+bk)
  differs from (q+bq)dot k by a j-constant). bv is folded into the
  output bias on the host: softmax weights sum to 1, so ctx_hat =
  ctx_norm + bv and bo' = bo + Wo @ bv. K/V projections become pure
  matmuls (PSUM->SBUF copies on ACT during the prologue, when ACT is
  otherwise idle).
- Scores matmuls are emitted in head PAIRS, interleaved A,B,A,B. Head
  h sits at partition base (h%2)*64 of d-tile h//2, so the pair's
  lhsT tiles land on PE row-groups (0,0)/(64,0): HW-measured 468 vs
  771 ns/matmul (the row-group alternation lets the PE pull the next
  LDWEIGHTS ahead of the in-flight matmul).
- Softmax denominators: ones column appended to V, so the ctx matmul
  emits [ctx; den] per head. 1/den goes (partition-shifted, per head)
  into the base-0 row block rflat; a deferred K=1 ones-matmul fans it
  across partitions into PSUM and one DVE multiply normalizes in
  place. No gpsimd: partition_broadcast costs ~us of Q7 ucode per op
  on HW and interleaving gpsimd op types forces 73
  PseudoReloadLibraryIndex library reloads (the all-gpsimd norm
  measured 831us end-to-end vs 319us for this scheme); a broadcast
  DMA measures slower too (serializes SP dispatch vs output stores).
- Out-projection of chunk ch is interleaved into chunk ch+1's head
  pairs as PE filler.
"""
import sys
sys.path.insert(0, "/opt/trn_rl_repo")
import numpy as np

import concourse.bacc as bacc
import concourse.tile as tile
from concourse import mybir
from concourse.bass_utils import run_bass_kernel_spmd

B, S, D, H, DIL = 4, 4096, 768, 12, 8
HD = D // H            # 64
SD = S // DIL          # 512 dilated K/V positions
NCORE = 8
TOK = B * S // NCORE   # 2048 query tokens per core
TCH = 512              # chunk of query tokens processed at once
NCH = TOK // TCH       # 4
NKT = D // 128         # 6 contraction tiles
NJT = SD // 128        # 4 j tiles
F32 = mybir.dt.float32
BF16 = mybir.dt.bfloat16
SCALE = 1.0 / float(np.sqrt(HD))
EXP = mybir.ActivationFunctionType.Exp
ADD = mybir.AluOpType.add
MULT = mybir.AluOpType.mult

_CACHE = {}


def _hs(t, h, cols):
    """[64, ...] head slice of a [128, NKT, T] d-on-partition tensor."""
    base = (h % 2) * HD
    return t[base:base + HD, h // 2, cols]


def _build(repeat=1, ablate=""):
    nc = bacc.Bacc("TRN2", target_bir_lowering=False, debug=False,
                   num_devices=NCORE)

    xT_d = nc.dram_tensor("xT", [D, TOK], BF16, kind="ExternalInput")
    xdT_d = nc.dram_tensor("xdT", [D, SD], BF16, kind="ExternalInput")
    w_d = {n: nc.dram_tensor(n, [D, D], BF16, kind="ExternalInput")
           for n in ("wqT", "wkT", "wvT", "woT")}
    bq_d = nc.dram_tensor("bq", [D], F32, kind="ExternalInput")
    bo_d = nc.dram_tensor("bo", [D], F32, kind="ExternalInput")
    out_d = nc.dram_tensor("out", [TOK, D], F32, kind="ExternalOutput")

    from contextlib import ExitStack
    with tile.TileContext(nc) as tc, ExitStack() as es:
        cpool = es.enter_context(tc.tile_pool(name="const", bufs=1))
        xpool = es.enter_context(tc.tile_pool(name="xin", bufs=2))
        qpool = es.enter_context(tc.tile_pool(name="qt", bufs=2))
        epool = es.enter_context(tc.tile_pool(name="exps", bufs=4))
        ctpool = es.enter_context(tc.tile_pool(name="ctxt", bufs=2))
        opool = es.enter_context(tc.tile_pool(name="outs", bufs=2))
        fpool = es.enter_context(tc.tile_pool(name="flat", bufs=2))
        prj = es.enter_context(tc.tile_pool(name="prj", bufs=2, space="PSUM"))
        scp = es.enter_context(tc.tile_pool(name="sc", bufs=2, space="PSUM"))
        cxp = es.enter_context(tc.tile_pool(name="cx", bufs=2, space="PSUM"))

        # ---- constants: weights / biases (issue order = first-use order) ----
        w_sb = {n: cpool.tile([128, NKT, D], BF16, name=n)
                for n in ("wkT", "wvT", "wqT", "woT")}
        xdT_sb = cpool.tile([128, NKT, SD], BF16, name="xdT")
        _wk_r = w_d["wkT"].rearrange("(ko p) o -> p ko o", p=128)
        _xd_r = xdT_d.rearrange("(ko p) j -> p ko j", p=128)
        nc.sync.dma_start(w_sb["wkT"][:, 0:1], _wk_r[:, 0:1])
        nc.sync.dma_start(xdT_sb[:, 0:1], _xd_r[:, 0:1])
        nc.sync.dma_start(w_sb["wkT"][:, 1:2], _wk_r[:, 1:2])
        nc.sync.dma_start(xdT_sb[:, 1:2], _xd_r[:, 1:2])
        nc.sync.dma_start(w_sb["wkT"][:, 2:NKT], _wk_r[:, 2:NKT])
        nc.sync.dma_start(xdT_sb[:, 2:NKT], _xd_r[:, 2:NKT])
        nc.sync.dma_start(w_sb["wvT"][:],
                          w_d["wvT"].rearrange("(ko p) o -> p ko o", p=128))
        nc.sync.dma_start(w_sb["wqT"][:],
                          w_d["wqT"].rearrange("(ko p) o -> p ko o", p=128))
        bq_sb = cpool.tile([128, NKT], F32, name="bq")
        nc.sync.dma_start(bq_sb[:], bq_d.rearrange("(ko p) -> p ko", p=128))
        nc.sync.dma_start(w_sb["woT"][:],
                          w_d["woT"].rearrange("(ko p) o -> p ko o", p=128))
        bo_sb = cpool.tile([128, D], F32, name="bo")
        nc.sync.dma_start(bo_sb[:], bo_d[None, :].to_broadcast((128, D)))
        ones_sb = cpool.tile([1, 128], BF16, name="ones")
        nc.vector.memset(ones_sb[:], 1.0)

        for _rep in range(repeat):
            # ---- K^T [d, j] (pure matmul; ACT copies while ACT is idle) ----
            kT_sb = cpool.tile([128, NKT, SD], BF16, name="kT")
            for m in range(NKT):
                ps = prj.tile([128, 512], F32, name="prjps")
                for kt in range(NKT):
                    nc.tensor.matmul(ps[:], w_sb["wkT"][:, kt, m * 128:(m + 1) * 128],
                                     xdT_sb[:, kt, :], start=(kt == 0),
                                     stop=(kt == NKT - 1))
                nc.scalar.copy(kT_sb[:, m, :], ps[:])

            # ---- V [j, head, 64|1] bf16 (ones col -> denominator row) ----
            v_sb = cpool.tile([128, NJT, H, HD + 1], BF16, name="v")
            nc.vector.memset(v_sb[:, :, :, HD:], 1.0)
            for jt in range(NJT):
                for nh0, nh1 in ((0, 8), (8, 12)):
                    ncols = (nh1 - nh0) * HD
                    ps = prj.tile([128, 512], F32, name="prjps")
                    for kt in range(NKT):
                        nc.tensor.matmul(ps[:, :ncols],
                                         xdT_sb[:, kt, jt * 128:(jt + 1) * 128],
                                         w_sb["wvT"][:, kt, nh0 * HD:nh1 * HD],
                                         start=(kt == 0), stop=(kt == NKT - 1))
                    nc.scalar.copy(
                        v_sb[:, jt, nh0:nh1, :HD],
                        ps[:, :ncols].rearrange("p (h e) -> p h e", e=HD))

            def emit_qproj(xT_sb, qT_sb):
                for m in range(NKT):
                    ps = prj.tile([128, 512], F32, name="prjps")
                    for kt in range(NKT):
                        nc.tensor.matmul(ps[:], w_sb["wqT"][:, kt, m * 128:(m + 1) * 128],
                                         xT_sb[:, kt, :], start=(kt == 0),
                                         stop=(kt == NKT - 1))
                    nc.vector.tensor_tensor(
                        qT_sb[:, m, :], ps[:],
                        bq_sb[:, m, None].to_broadcast((128, TCH)), ADD)

            def emit_spair(qT_sb, hA, half, expA, expB):
                """Interleaved score MMs for heads hA, hA+1 (PE row packing),
                then the two exps."""
                spA = scp.tile([128, 2, TCH], F32, name="sp")
                spB = scp.tile([128, 2, TCH], F32, name="sp")
                for j2 in range(2):
                    jt = half * 2 + j2
                    jsl = slice(jt * 128, (jt + 1) * 128)
                    nc.tensor.matmul(spA[:, j2, :], _hs(kT_sb, hA, jsl),
                                     _hs(qT_sb, hA, slice(None)),
                                     start=True, stop=True)
                    nc.tensor.matmul(spB[:, j2, :], _hs(kT_sb, hA + 1, jsl),
                                     _hs(qT_sb, hA + 1, slice(None)),
                                     start=True, stop=True)
                hsl = slice(half * 2, half * 2 + 2)
                nc.scalar.activation(expA[:, hsl, :], spA[:], EXP, scale=SCALE)
                nc.scalar.activation(expB[:, hsl, :], spB[:], EXP, scale=SCALE)

            norm_q = []   # heads whose reciprocal is ready, broadcast pending

            def emit_ctx(ctxT_sb, rflat, exp_sb, h):
                """Unnormalized ctx for head h: the ones column of V makes
                PSUM row 64 the softmax denominator; its reciprocal lands
                (partition-shifted) in the base-0 row block rflat. The
                normalize multiply is deferred (norm_q) so the PE broadcast
                never waits on the reciprocal."""
                cps = cxp.tile([128, TCH], F32, name="cxps")
                for jt in range(NJT):
                    nc.tensor.matmul(cps[:HD + 1, :], v_sb[:, jt, h, :],
                                     exp_sb[:, jt, :], start=(jt == 0),
                                     stop=(jt == NJT - 1))
                sl = _hs(ctxT_sb, h, slice(None))
                nc.vector.tensor_copy(sl, cps[:HD, :])
                if ablate == "nonorm":
                    return
                with nc.allow_low_precision(reason="1/den in bf16: softmax "
                                            "denominator scale, ~0.4% rel"):
                    nc.vector.reciprocal(rflat[0:1, h, :], cps[HD:HD + 1, :])
                norm_q.append((ctxT_sb, rflat, h))

            def pop_norm(n):
                """Broadcast 1/den across partitions with a K=1 ones-matmul
                (gpsimd.partition_broadcast costs ~us of ucode per op on HW
                and thrashes the gpsimd library between op types; a
                broadcast DMA measures slower still — it serializes the SP
                dispatch queue against the output stores), then one DVE
                multiply straight from PSUM. Deferred >=1 slot so the PE
                matmul never waits on the reciprocal."""
                for _ in range(min(n, len(norm_q))):
                    ctxT_sb, rflat, h = norm_q.pop(0)
                    rbc = cxp.tile([128, TCH], F32, name="cxps")
                    nc.tensor.matmul(rbc[:], ones_sb[0:1, :],
                                     rflat[0:1, h, :], start=True, stop=True)
                    base = (h % 2) * HD
                    sl = _hs(ctxT_sb, h, slice(None))
                    nc.vector.tensor_tensor(sl, sl, rbc[base:base + HD, :],
                                            MULT)

            def emit_out_grp(ctxT_sb, o_sb, ch, tt, g):
                n0, n1 = ((0, 512), (512, 768))[g]
                ps = prj.tile([128, 512], F32, name="prjps")
                for kt in range(NKT):
                    nc.tensor.matmul(ps[:, :n1 - n0],
                                     ctxT_sb[:, kt, tt * 128:(tt + 1) * 128],
                                     w_sb["woT"][:, kt, n0:n1],
                                     start=(kt == 0), stop=(kt == NKT - 1))
                nc.vector.tensor_tensor(o_sb[:, n0:n1], ps[:, :n1 - n0],
                                        bo_sb[:, n0:n1], ADD)
                nc.sync.dma_start(
                    out_d[ch * TCH + tt * 128:ch * TCH + (tt + 1) * 128, n0:n1],
                    o_sb[:, n0:n1])

            # ---- chunk pipeline ----
            pending = []   # out-proj units of the previous chunk

            def pop_out(n):
                for _ in range(min(n, len(pending))):
                    pending.pop(0)()

            for ch in range(NCH):
                tsl = slice(ch * TCH, (ch + 1) * TCH)
                xT_sb = xpool.tile([128, NKT, TCH], BF16, name="xT")
                nc.sync.dma_start(
                    xT_sb[:],
                    xT_d.rearrange("(ko p) t -> p ko t", p=128)[:, :, tsl])

                qT_sb = qpool.tile([128, NKT, TCH], BF16, name="qT")
                emit_qproj(xT_sb, qT_sb)

                ctxT_sb = ctpool.tile([128, NKT, TCH], BF16, name="ctxT")
                rflat = fpool.tile([1, H, TCH], BF16, name="rflat")
                prev_pair = None   # (hA, expA, expB) awaiting ctx
                for p in range(H // 2):
                    hA = 2 * p
                    expA = epool.tile([128, NJT, TCH], BF16, name="exp")
                    expB = epool.tile([128, NJT, TCH], BF16, name="exp")
                    emit_spair(qT_sb, hA, 0, expA, expB)
                    if prev_pair is not None:
                        emit_ctx(ctxT_sb, rflat, prev_pair[1], prev_pair[0])
                    if len(norm_q) >= 2:
                        pop_norm(1)
                    if p >= 2:
                        pop_out(1)
                    emit_spair(qT_sb, hA, 1, expA, expB)
                    if prev_pair is not None:
                        emit_ctx(ctxT_sb, rflat, prev_pair[2],
                                 prev_pair[0] + 1)
                    if len(norm_q) >= 2:
                        pop_norm(1)
                    if p >= 2:
                        pop_out(1)
                    prev_pair = (hA, expA, expB)
                emit_ctx(ctxT_sb, rflat, prev_pair[1], prev_pair[0])
                pop_norm(1)
                emit_ctx(ctxT_sb, rflat, prev_pair[2], prev_pair[0] + 1)
                pop_norm(len(norm_q))
                pop_out(8)
                # queue this chunk's out-projection; popped during the next
                # chunk's pairs (norm multiplies drain on Pool meanwhile)
                o_tiles = [opool.tile([128, D], F32, name="osb")
                           for _ in range(4)]
                pending = [
                    (lambda c=ctxT_sb, o=o_tiles[tt], cc=ch, t=tt, g=g:
                     emit_out_grp(c, o, cc, t, g))
                    for tt in range(4) for g in range(2)]

            # tail: last chunk's out-projection
            pop_out(8)

    nc.compile()
    return nc


def _get_nc(repeat=1, ablate=""):
    key = (repeat, ablate)
    if key not in _CACHE:
        _CACHE[key] = _build(repeat, ablate)
    return _CACHE[key]


def make_in_maps(x, Wq, bq, Wk, bk, Wv, bv, Wo, bo):
    import ml_dtypes
    bf = ml_dtypes.bfloat16
    wqT = np.ascontiguousarray(np.asarray(Wq, np.float32).T).astype(bf)
    wkT = np.ascontiguousarray(np.asarray(Wk, np.float32).T).astype(bf)
    wvT = np.ascontiguousarray(np.asarray(Wv, np.float32).T).astype(bf)
    woT = np.ascontiguousarray(np.asarray(Wo, np.float32).T).astype(bf)
    bq = np.asarray(bq, np.float32)
    # bk: dropped (softmax shift invariance). bv: folded into bo since the
    # normalized softmax weights sum to 1 -> out += (Wo @ bv).
    bo_adj = np.asarray(bo, np.float32) + (
        np.asarray(Wo, np.float32) @ np.asarray(bv, np.float32))
    x = np.asarray(x, np.float32)
    in_maps = []
    for c in range(NCORE):
        b, half = divmod(c, 2)
        xT = np.ascontiguousarray(x[b, half * TOK:(half + 1) * TOK, :].T).astype(bf)
        xdT = np.ascontiguousarray(x[b, ::DIL, :].T).astype(bf)
        in_maps.append({
            "xT": xT, "xdT": xdT,
            "wqT": wqT, "wkT": wkT, "wvT": wvT, "woT": woT,
            "bq": bq, "bo": bo_adj,
        })
    return in_maps


def assemble(results):
    out = np.empty((B, S, D), np.float32)
    for c in range(NCORE):
        b, half = divmod(c, 2)
        out[b, half * TOK:(half + 1) * TOK, :] = results[c]["out"]
    return out


def kernel(**inputs):
    nc = _get_nc()
    in_maps = make_in_maps(**inputs)
    res = run_bass_kernel_spmd(nc, in_maps, core_ids=list(range(NCORE)))
    return assemble(res.results)


# revision 24
# speedup vs baseline: 1.0544x; 1.0544x over previous
"""Dilated attention (B=4,S=4096,D=768,H=12,DIL=8) on 8 TRN2 NeuronCores.

Sharding: batch x seq-half data-parallel -> core c handles batch c//2,
sequence half c%2 (2048 query tokens). The DIL-strided K/V positions
(512 per batch) are position-independent, so each core computes K/V for
its batch's 512 dilated positions locally.

V2 design notes (vs the f32r baseline; HW-measured 496us -> 319us per
iteration under the chained-repeat burst-slope protocol of hwtime.py):
- All matmul operands are bf16 (host-cast): measured identical PE rate
  to f32r (616 vs 614 ns per [128,512] matmul) at half the HBM traffic
  and SBUF footprint; f32 PSUM accumulation keeps precision (end-to-end
  rel err 3.2e-3 vs the 2e-2 gate).
- bk is dropped entirely (softmax shift invariance: (q+bq)dot(k+bk)
  differs from (q+bq)dot k by a j-constant). bv is folded into the
  output bias on the host: softmax weights sum to 1, so ctx_hat =
  ctx_norm + bv and bo' = bo + Wo @ bv. K/V projections become pure
  matmuls (PSUM->SBUF copies on ACT during the prologue, when ACT is
  otherwise idle).
- Scores matmuls are emitted in head PAIRS, interleaved A,B,A,B. Head
  h sits at partition base (h%2)*64 of d-tile h//2, so the pair's
  lhsT tiles land on PE row-groups (0,0)/(64,0): HW-measured 468 vs
  771 ns/matmul (the row-group alternation lets the PE pull the next
  LDWEIGHTS ahead of the in-flight matmul).
- Softmax denominators: ones column appended to V, so the ctx matmul
  emits [ctx; den] per head. 1/den goes (partition-shifted, per head)
  into the base-0 row block rflat; a deferred K=1 ones-matmul fans it
  across partitions into PSUM and one DVE multiply normalizes in
  place. No gpsimd: partition_broadcast costs ~us of Q7 ucode per op
  on HW and interleaving gpsimd op types forces 73
  PseudoReloadLibraryIndex library reloads (the all-gpsimd norm
  measured 831us end-to-end vs 319us for this scheme); a broadcast
  DMA measures slower too (serializes SP dispatch vs output stores).
- Out-projection of chunk ch is interleaved into chunk ch+1's head
  pairs as PE filler.
"""
import sys
sys.path.insert(0, "/opt/trn_rl_repo")
import numpy as np

import concourse.bacc as bacc
import concourse.tile as tile
from concourse import mybir
from concourse.bass_utils import run_bass_kernel_spmd

B, S, D, H, DIL = 4, 4096, 768, 12, 8
HD = D // H            # 64
SD = S // DIL          # 512 dilated K/V positions
NCORE = 8
TOK = B * S // NCORE   # 2048 query tokens per core
TCH = 512              # chunk of query tokens processed at once
NCH = TOK // TCH       # 4
NKT = D // 128         # 6 contraction tiles
NJT = SD // 128        # 4 j tiles
F32 = mybir.dt.float32
BF16 = mybir.dt.bfloat16
SCALE = 1.0 / float(np.sqrt(HD))
EXP = mybir.ActivationFunctionType.Exp
ADD = mybir.AluOpType.add
MULT = mybir.AluOpType.mult

_CACHE = {}


def _hs(t, h, cols):
    """[64, ...] head slice of a [128, NKT, T] d-on-partition tensor."""
    base = (h % 2) * HD
    return t[base:base + HD, h // 2, cols]


def _build(repeat=1, ablate=""):
    nc = bacc.Bacc("TRN2", target_bir_lowering=False, debug=False,
                   num_devices=NCORE)

    xT_d = nc.dram_tensor("xT", [D, TOK], BF16, kind="ExternalInput")
    xdT_d = nc.dram_tensor("xdT", [D, SD], BF16, kind="ExternalInput")
    w_d = {n: nc.dram_tensor(n, [D, D], BF16, kind="ExternalInput")
           for n in ("wqT", "wkT", "wvT", "woT")}
    bq_d = nc.dram_tensor("bq", [D], F32, kind="ExternalInput")
    bo_d = nc.dram_tensor("bo", [D], F32, kind="ExternalInput")
    out_d = nc.dram_tensor("out", [TOK, D], F32, kind="ExternalOutput")

    from contextlib import ExitStack
    with tile.TileContext(nc) as tc, ExitStack() as es:
        cpool = es.enter_context(tc.tile_pool(name="const", bufs=1))
        xpool = es.enter_context(tc.tile_pool(name="xin", bufs=2))
        qpool = es.enter_context(tc.tile_pool(name="qt", bufs=2))
        epool = es.enter_context(tc.tile_pool(name="exps", bufs=4))
        ctpool = es.enter_context(tc.tile_pool(name="ctxt", bufs=2))
        opool = es.enter_context(tc.tile_pool(name="outs", bufs=2))
        fpool = es.enter_context(tc.tile_pool(name="flat", bufs=2))
        prj = es.enter_context(tc.tile_pool(name="prj", bufs=2, space="PSUM"))
        scp = es.enter_context(tc.tile_pool(name="sc", bufs=2, space="PSUM"))
        cxp = es.enter_context(tc.tile_pool(name="cx", bufs=2, space="PSUM"))

        # ---- constants: weights / biases (issue order = first-use order) ----
        w_sb = {n: cpool.tile([128, NKT, D], BF16, name=n)
                for n in ("wkT", "wvT", "wqT", "woT")}
        xdT_sb = cpool.tile([128, NKT, SD], BF16, name="xdT")
        _wk_r = w_d["wkT"].rearrange("(ko p) o -> p ko o", p=128)
        _xd_r = xdT_d.rearrange("(ko p) j -> p ko j", p=128)
        nc.sync.dma_start(w_sb["wkT"][:, 0:1], _wk_r[:, 0:1])
        nc.sync.dma_start(xdT_sb[:, 0:1], _xd_r[:, 0:1])
        nc.sync.dma_start(w_sb["wkT"][:, 1:2], _wk_r[:, 1:2])
        nc.sync.dma_start(xdT_sb[:, 1:2], _xd_r[:, 1:2])
        nc.sync.dma_start(w_sb["wkT"][:, 2:NKT], _wk_r[:, 2:NKT])
        nc.sync.dma_start(xdT_sb[:, 2:NKT], _xd_r[:, 2:NKT])
        nc.sync.dma_start(w_sb["wvT"][:],
                          w_d["wvT"].rearrange("(ko p) o -> p ko o", p=128))
        nc.sync.dma_start(w_sb["wqT"][:],
                          w_d["wqT"].rearrange("(ko p) o -> p ko o", p=128))
        bq_sb = cpool.tile([128, NKT], F32, name="bq")
        nc.sync.dma_start(bq_sb[:], bq_d.rearrange("(ko p) -> p ko", p=128))
        nc.sync.dma_start(w_sb["woT"][:],
                          w_d["woT"].rearrange("(ko p) o -> p ko o", p=128))
        bo_sb = cpool.tile([128, D], F32, name="bo")
        nc.sync.dma_start(bo_sb[:], bo_d[None, :].to_broadcast((128, D)))
        ones_sb = cpool.tile([1, 128], BF16, name="ones")
        nc.vector.memset(ones_sb[:], 1.0)

        if ablate != "nopad":
            # zero-padded per-head K tiles: scores lhsT becomes a full-width
            # [128,128] block (head rows + exact zeros) to test whether the
            # HAM clock gate holds the 2.4GHz p-state for full-row matmuls
            kTp_sb = cpool.tile([128, H, SD], BF16, name="kTp")
            nc.vector.memset(kTp_sb[:], 0.0)

        for _rep in range(repeat):
            # ---- K^T [d, j] (pure matmul; ACT copies while ACT is idle) ----
            kT_sb = cpool.tile([128, NKT, SD], BF16, name="kT")
            for m in range(NKT):
                ps = prj.tile([128, 512], F32, name="prjps")
                for kt in range(NKT):
                    nc.tensor.matmul(ps[:], w_sb["wkT"][:, kt, m * 128:(m + 1) * 128],
                                     xdT_sb[:, kt, :], start=(kt == 0),
                                     stop=(kt == NKT - 1))
                nc.scalar.copy(kT_sb[:, m, :], ps[:])
                if ablate != "nopad":
                    for hh in range(2):
                        h = 2 * m + hh
                        b0 = hh * HD
                        nc.scalar.copy(kTp_sb[b0:b0 + HD, h, :],
                                       ps[b0:b0 + HD, :])

            # ---- V [j, head, 64|1] bf16 (ones col -> denominator row) ----
            v_sb = cpool.tile([128, NJT, H, HD + 1], BF16, name="v")
            nc.vector.memset(v_sb[:, :, :, HD:], 1.0)
            for jt in range(NJT):
                for nh0, nh1 in ((0, 8), (8, 12)):
                    ncols = (nh1 - nh0) * HD
                    ps = prj.tile([128, 512], F32, name="prjps")
                    for kt in range(NKT):
                        nc.tensor.matmul(ps[:, :ncols],
                                         xdT_sb[:, kt, jt * 128:(jt + 1) * 128],
                                         w_sb["wvT"][:, kt, nh0 * HD:nh1 * HD],
                                         start=(kt == 0), stop=(kt == NKT - 1))
                    nc.scalar.copy(
                        v_sb[:, jt, nh0:nh1, :HD],
                        ps[:, :ncols].rearrange("p (h e) -> p h e", e=HD))

            def emit_qproj(xT_sb, qT_sb):
                for m in range(NKT):
                    ps = prj.tile([128, 512], F32, name="prjps")
                    for kt in range(NKT):
                        nc.tensor.matmul(ps[:], w_sb["wqT"][:, kt, m * 128:(m + 1) * 128],
                                         xT_sb[:, kt, :], start=(kt == 0),
                                         stop=(kt == NKT - 1))
                    nc.vector.tensor_tensor(
                        qT_sb[:, m, :], ps[:],
                        bq_sb[:, m, None].to_broadcast((128, TCH)), ADD)

            def emit_spair(qT_sb, hA, half, expA, expB):
                """Interleaved score MMs for heads hA, hA+1 (PE row packing),
                then the two exps."""
                spA = scp.tile([128, 2, TCH], F32, name="sp")
                spB = scp.tile([128, 2, TCH], F32, name="sp")
                for j2 in range(2):
                    jt = half * 2 + j2
                    jsl = slice(jt * 128, (jt + 1) * 128)
                    if ablate != "nopad":
                        nc.tensor.matmul(spA[:, j2, :], kTp_sb[:, hA, jsl],
                                         qT_sb[:, hA // 2, :],
                                         start=True, stop=True)
                        nc.tensor.matmul(spB[:, j2, :], kTp_sb[:, hA + 1, jsl],
                                         qT_sb[:, hA // 2, :],
                                         start=True, stop=True)
                        continue
                    nc.tensor.matmul(spA[:, j2, :], _hs(kT_sb, hA, jsl),
                                     _hs(qT_sb, hA, slice(None)),
                                     start=True, stop=True)
                    nc.tensor.matmul(spB[:, j2, :], _hs(kT_sb, hA + 1, jsl),
                                     _hs(qT_sb, hA + 1, slice(None)),
                                     start=True, stop=True)
                hsl = slice(half * 2, half * 2 + 2)
                nc.scalar.activation(expA[:, hsl, :], spA[:], EXP, scale=SCALE)
                nc.scalar.activation(expB[:, hsl, :], spB[:], EXP, scale=SCALE)

            norm_q = []   # heads whose reciprocal is ready, broadcast pending

            def emit_ctx(ctxT_sb, rflat, exp_sb, h):
                """Unnormalized ctx for head h: the ones column of V makes
                PSUM row 64 the softmax denominator; its reciprocal lands
                (partition-shifted) in the base-0 row block rflat. The
                normalize multiply is deferred (norm_q) so the PE broadcast
                never waits on the reciprocal."""
                cps = cxp.tile([128, TCH], F32, name="cxps")
                for jt in range(NJT):
                    nc.tensor.matmul(cps[:HD + 1, :], v_sb[:, jt, h, :],
                                     exp_sb[:, jt, :], start=(jt == 0),
                                     stop=(jt == NJT - 1))
                sl = _hs(ctxT_sb, h, slice(None))
                nc.vector.tensor_copy(sl, cps[:HD, :])
                if ablate == "nonorm":
                    return
                with nc.allow_low_precision(reason="1/den in bf16: softmax "
                                            "denominator scale, ~0.4% rel"):
                    nc.vector.reciprocal(rflat[0:1, h, :], cps[HD:HD + 1, :])
                norm_q.append((ctxT_sb, rflat, h))

            def pop_norm(n):
                """Broadcast 1/den across partitions with a K=1 ones-matmul
                (gpsimd.partition_broadcast costs ~us of ucode per op on HW
                and thrashes the gpsimd library between op types; a
                broadcast DMA measures slower still — it serializes the SP
                dispatch queue against the output stores), then one DVE
                multiply straight from PSUM. Deferred >=1 slot so the PE
                matmul never waits on the reciprocal."""
                for _ in range(min(n, len(norm_q))):
                    ctxT_sb, rflat, h = norm_q.pop(0)
                    rbc = cxp.tile([128, TCH], F32, name="cxps")
                    nc.tensor.matmul(rbc[:], ones_sb[0:1, :],
                                     rflat[0:1, h, :], start=True, stop=True)
                    base = (h % 2) * HD
                    sl = _hs(ctxT_sb, h, slice(None))
                    nc.vector.tensor_tensor(sl, sl, rbc[base:base + HD, :],
                                            MULT)

            def emit_out_grp(ctxT_sb, o_sb, ch, tt, g):
                n0, n1 = ((0, 512), (512, 768))[g]
                ps = prj.tile([128, 512], F32, name="prjps")
                for kt in range(NKT):
                    nc.tensor.matmul(ps[:, :n1 - n0],
                                     ctxT_sb[:, kt, tt * 128:(tt + 1) * 128],
                                     w_sb["woT"][:, kt, n0:n1],
                                     start=(kt == 0), stop=(kt == NKT - 1))
                nc.vector.tensor_tensor(o_sb[:, n0:n1], ps[:, :n1 - n0],
                                        bo_sb[:, n0:n1], ADD)
                nc.sync.dma_start(
                    out_d[ch * TCH + tt * 128:ch * TCH + (tt + 1) * 128, n0:n1],
                    o_sb[:, n0:n1])

            # ---- chunk pipeline ----
            pending = []   # out-proj units of the previous chunk

            def pop_out(n):
                for _ in range(min(n, len(pending))):
                    pending.pop(0)()

            for ch in range(NCH):
                tsl = slice(ch * TCH, (ch + 1) * TCH)
                xT_sb = xpool.tile([128, NKT, TCH], BF16, name="xT")
                nc.sync.dma_start(
                    xT_sb[:],
                    xT_d.rearrange("(ko p) t -> p ko t", p=128)[:, :, tsl])

                qT_sb = qpool.tile([128, NKT, TCH], BF16, name="qT")
                emit_qproj(xT_sb, qT_sb)

                ctxT_sb = ctpool.tile([128, NKT, TCH], BF16, name="ctxT")
                rflat = fpool.tile([1, H, TCH], BF16, name="rflat")
                prev_pair = None   # (hA, expA, expB) awaiting ctx
                for p in range(H // 2):
                    hA = 2 * p
                    expA = epool.tile([128, NJT, TCH], BF16, name="exp")
                    expB = epool.tile([128, NJT, TCH], BF16, name="exp")
                    emit_spair(qT_sb, hA, 0, expA, expB)
                    if prev_pair is not None:
                        emit_ctx(ctxT_sb, rflat, prev_pair[1], prev_pair[0])
                    if len(norm_q) >= 2:
                        pop_norm(1)
                    if p >= 2:
                        pop_out(1)
                    emit_spair(qT_sb, hA, 1, expA, expB)
                    if prev_pair is not None:
                        emit_ctx(ctxT_sb, rflat, prev_pair[2],
                                 prev_pair[0] + 1)
                    if len(norm_q) >= 2:
                        pop_norm(1)
                    if p >= 2:
                        pop_out(1)
                    prev_pair = (hA, expA, expB)
                emit_ctx(ctxT_sb, rflat, prev_pair[1], prev_pair[0])
                pop_norm(1)
                emit_ctx(ctxT_sb, rflat, prev_pair[2], prev_pair[0] + 1)
                pop_norm(len(norm_q))
                pop_out(8)
                # queue this chunk's out-projection; popped during the next
                # chunk's pairs (norm multiplies drain on Pool meanwhile)
                o_tiles = [opool.tile([128, D], F32, name="osb")
                           for _ in range(4)]
                pending = [
                    (lambda c=ctxT_sb, o=o_tiles[tt], cc=ch, t=tt, g=g:
                     emit_out_grp(c, o, cc, t, g))
                    for tt in range(4) for g in range(2)]

            # tail: last chunk's out-projection
            pop_out(8)

    nc.compile()
    return nc


def _get_nc(repeat=1, ablate=""):
    key = (repeat, ablate)
    if key not in _CACHE:
        _CACHE[key] = _build(repeat, ablate)
    return _CACHE[key]


def make_in_maps(x, Wq, bq, Wk, bk, Wv, bv, Wo, bo):
    import ml_dtypes
    bf = ml_dtypes.bfloat16
    wqT = np.ascontiguousarray(np.asarray(Wq, np.float32).T).astype(bf)
    wkT = np.ascontiguousarray(np.asarray(Wk, np.float32).T).astype(bf)
    wvT = np.ascontiguousarray(np.asarray(Wv, np.float32).T).astype(bf)
    woT = np.ascontiguousarray(np.asarray(Wo, np.float32).T).astype(bf)
    bq = np.asarray(bq, np.float32)
    # bk: dropped (softmax shift invariance). bv: folded into bo since the
    # normalized softmax weights sum to 1 -> out += (Wo @ bv).
    bo_adj = np.asarray(bo, np.float32) + (
        np.asarray(Wo, np.float32) @ np.asarray(bv, np.float32))
    x = np.asarray(x, np.float32)
    in_maps = []
    for c in range(NCORE):
        b, half = divmod(c, 2)
        xT = np.ascontiguousarray(x[b, half * TOK:(half + 1) * TOK, :].T).astype(bf)
        xdT = np.ascontiguousarray(x[b, ::DIL, :].T).astype(bf)
        in_maps.append({
            "xT": xT, "xdT": xdT,
            "wqT": wqT, "wkT": wkT, "wvT": wvT, "woT": woT,
            "bq": bq, "bo": bo_adj,
        })
    return in_maps


def assemble(results):
    out = np.empty((B, S, D), np.float32)
    for c in range(NCORE):
        b, half = divmod(c, 2)
        out[b, half * TOK:(half + 1) * TOK, :] = results[c]["out"]
    return out


def kernel(**inputs):
    nc = _get_nc()
    in_maps = make_in_maps(**inputs)
    res = run_bass_kernel_spmd(nc, in_maps, core_ids=list(range(NCORE)))
    return assemble(res.results)
